# revision 1
# baseline (speedup 1.0000x reference)
"""Trainium2 Bass kernel for CubPL2d persistence-landscape problem.

Computes, for full inputs
    x:         [128, 64, 64, 64] f32
    birth_idx: [128, 64, 128] int
    death_idx: [128, 64, 128] int
    pair_dim:  [128, 64, 128] int
the output [128, 64, 2, 2, 32] f32:
    tri[b,c,p,t] = max(min(t_seq[t] - x[b,c,birth], x[b,c,death] - t_seq[t]), 0)
    out[b,c,d,k,t] = k-th largest over p of (tri where pair_dim==d else 0)

Sharding: pure data-parallel over batch dim B across 8 cores (16 batches each).

Per-core algorithm (BC = 16*64 = 1024 (b,c) rows, blocks of 128 rows):
  - the per-(b,c) gather xf[birth/death_idx] runs on GPSIMD as ONE
    local_scatter per block: the host re-encodes the index tensors as an
    inverse table T[pos] = slot (slot 0..127 = birth pair, 128..255 = death
    pair, -1 = unreferenced; one winner per position), and the Q7 cores
    stream the x row (fp16) + T and scatter x[pos] -> G[T[pos]].  This is
    a sequential-stream scan (~8 cyc per 2 positions, 16 partitions in
    parallel) instead of ap_gather's one non-pipelined RD_CMD per 2
    indices, which was 82% of the baseline runtime.
  - positions referenced by k>1 slots have one winner in T; the remaining
    slots are filled EXACTLY by one more tiny local_scatter whose feed
    arrays (host-precomputed, rank r=1..3 member of each duplicate class,
    sourced from the winner slot) scatter G[winner] -> G[rank-r member];
    max-merge (values >= 0) applies them.  Seed-0 data has max class size
    4, so 3 feed ranks suffice (exact for any data with <= 4-way position
    collisions per (b,c) row).
  - triangle construction in tent form: 2*tri = (d-b) - |2t - (b+d)|.
    The |.| runs on the (otherwise idle) scalar engine; the vector engine
    does only 3 large tensor_tensor ops per block (w = 2t - s, then
    tri{0,1} = h{0,1} - |w|) instead of 5.  The pair_dim mask is folded
    into the per-pair constant h as a -2 penalty (masked values <= -1.69
    can never displace a positive candidate, and negatives all relu to 0,
    matching the reference's where(mask, tri, 0) + top_k exactly).
  - per (dim, t) top-2 over pairs via InstMax (exact top-8 per partition
    row); final relu halves the doubled scale (relu(0.5 x) = 0.5 relu(x)).
  - the block loop is software-pipelined (block k's scatter issues before
    block k-1's fixup/compute) so GPSIMD never waits on scalar/vector.
"""

import numpy as np

import concourse.bass as bass
import concourse.bacc as bacc
import concourse.mybir as mybir
from concourse.tile import TileContext
from concourse.bass_utils import run_bass_kernel_spmd

T_MIN, T_MAX = 0.03, 0.34
STEPS = 32
K_MAX = 2
N_DIMS = 2
B, C, H, W = 128, 64, 64, 64
P = 128
P2 = 2 * P  # birth+death slots per row
HW = H * W
N_CORES = 8
B_LOC = B // N_CORES  # 16
BC_FULL = B_LOC * C  # 1024 (b,c) rows per core
N_RANKS = 3  # feed ranks handled (covers position-collision classes <= 4)

F32 = mybir.dt.float32
F16 = mybir.dt.float16
I16 = mybir.dt.int16
AF = mybir.ActivationFunctionType
ALU = mybir.AluOpType

COMPUTE_DT = F16  # fp16 keeps ~3.5 decimal digits; output scale ~0.34


def build_nc(bc: int = BC_FULL, cdt=COMPUTE_DT, repeat: int = 1,
             ablate: frozenset = frozenset()) -> bass.Bass:
    """Build the single-core Bass program for a shard with `bc` (b,c) rows.

    repeat > 1 wraps the whole block loop in a hardware For loop that redoes
    the (idempotent) computation `repeat` times — benchmarking only.
    ablate: subset of {"scatter", "chain", "construct", "max"} — skip those
    stages (outputs become garbage; timing-bisection only).
    """
    assert bc % 128 == 0
    nb = bc // 128
    tstep = (T_MAX - T_MIN) / (STEPS - 1)

    nc = bacc.Bacc(None, target_bir_lowering=False)
    x_t = nc.dram_tensor("x16", [bc, HW], F16, kind="ExternalInput")
    tinv_t = nc.dram_tensor("tinv", [bc, HW], I16, kind="ExternalInput")
    # feeds for ranks 1..N_RANKS (N_RANKS*P2) then pair_dim (P)
    aux_t = nc.dram_tensor("aux", [bc, N_RANKS * P2 + P], I16,
                           kind="ExternalInput")
    out_t = nc.dram_tensor("out", [bc, N_DIMS * K_MAX * STEPS], F32,
                           kind="ExternalOutput")

    with TileContext(nc) as tc:
        with (
            tc.tile_pool(name="const", bufs=1) as cpool,
            tc.tile_pool(name="xrows", bufs=2) as xpool,
            tc.tile_pool(name="idx", bufs=2) as ipool,
            tc.tile_pool(name="small", bufs=3) as spool,
            tc.tile_pool(name="big", bufs=2) as bpool,
            tc.tile_pool(name="psum", bufs=4, space="PSUM") as ppool,
        ):
            # t2_rep tile [128, STEPS, P]: 2*t replicated along p, so every
            # operand of the big tensor_tensor ops is packed in its last dim
            # (required for the DVE 2x_1p fp16 mode).
            neg1 = cpool.tile([128, 1], F32)
            nc.vector.memset(neg1[:, :], -1.0)
            # +I / -I fp16 identity matrices (PE stationaries for the
            # w = 2t - s accumulation into PSUM)
            rowi = cpool.tile([128, 128], F16)
            coli = cpool.tile([128, 128], F16)
            nc.gpsimd.iota(rowi[:, :], pattern=[[0, 128]], base=0,
                           channel_multiplier=1,
                           allow_small_or_imprecise_dtypes=True)
            nc.gpsimd.iota(coli[:, :], pattern=[[1, 128]], base=0,
                           channel_multiplier=0,
                           allow_small_or_imprecise_dtypes=True)
            ident = cpool.tile([128, 128], F16)
            nc.vector.tensor_tensor(out=ident[:, :], in0=rowi[:, :],
                                    in1=coli[:, :], op=ALU.is_equal)
            nident = cpool.tile([128, 128], F16)
            nc.vector.tensor_scalar(nident[:, :], ident[:, :], -1.0, None,
                                    op0=ALU.mult)
            t2_rep = cpool.tile([128, STEPS, P], cdt)
            nc.gpsimd.iota(t2_rep[:, :, :], pattern=[[1, STEPS], [0, P]],
                           base=0, channel_multiplier=0,
                           allow_small_or_imprecise_dtypes=True)
            nc.scalar.activation(t2_rep[:, :, :], t2_rep[:, :, :], AF.Copy,
                                 bias=float(2 * T_MIN), scale=float(2 * tstep))

            def stage_load(blk):
                """DMA loads + gather-as-scatter for one 128-row block."""
                r0 = blk * 128
                xrow = xpool.tile([128, HW], F16, tag="xrow")
                nc.sync.dma_start(out=xrow[:, :], in_=x_t[r0:r0 + 128, :])
                tinv = xpool.tile([128, HW], I16, tag="tinv")
                nc.sync.dma_start(out=tinv[:, :], in_=tinv_t[r0:r0 + 128, :])
                aux = ipool.tile([128, N_RANKS * P2 + P], I16, tag="aux")
                nc.sync.dma_start(out=aux[:, :], in_=aux_t[r0:r0 + 128, :])

                # G[T[pos]] = xrow[pos] per partition
                G = spool.tile([128, P2], cdt, tag="G")
                if "scatter" in ablate:
                    nc.gpsimd.memset(G[:, :], 0.25)
                else:
                    nc.gpsimd.local_scatter(
                        out_ap=G[:, :], data_ap=xrow[:, :],
                        idxs_ap=tinv[:, :], channels=128,
                        num_elems=P2, num_idxs=HW)
                return G, aux

            def stage_g3copy(G):
                """Broadcast-copy of G for the feed scatter (ACT engine,
                emitted at the tail of the index so it doesn't block the
                abs/relu work queued ahead of it)."""
                if "chain" in ablate:
                    return None
                G3 = spool.tile([128, N_RANKS, P2], cdt, tag="G3")
                nc.scalar.copy(
                    G3[:, :, :],
                    G[:, :].rearrange("p (r q) -> p r q", r=1)
                           .broadcast_to([128, N_RANKS, P2]))
                return G3

            def stage_fix(G, aux, G3):
                """Duplicate-position feed scatter (Pool, queued ahead of the
                next block's big scatter) + merge + per-pair constants."""
                pdim = aux[:, N_RANKS * P2:]

                if "chain" not in ablate:
                    Hs = spool.tile([128, P2], cdt, tag="Hs")
                    nc.gpsimd.local_scatter(
                        out_ap=Hs[:, :],
                        data_ap=G3[:, :, :],
                        idxs_ap=aux[:, 0:N_RANKS * P2],
                        channels=128, num_elems=P2,
                        num_idxs=N_RANKS * P2)
                    # Hs is zero off-target and G is zero exactly on the
                    # fed slots, so add == max-merge here.
                    nc.vector.tensor_tensor(out=G[:, :], in0=G[:, :],
                                            in1=Hs[:, :], op=ALU.add)

                births = G[:, :P]
                deaths = G[:, P:]

                # per-pair constants: s = b+d, hL0 = (d-b) - 2*pdim,
                # hL1 = (d-b) + 2*pdim.  The relative -2 separates masked
                # pairs; hL1's uniform +2 offset is removed by the final
                # relu bias (-1 on the halved scale).
                hp = spool.tile([128, 3, P], cdt, tag="hp")
                s_sum = hp[:, 0, :]
                hL0 = hp[:, 1, :]
                hL1 = hp[:, 2, :]
                nc.vector.tensor_tensor(out=s_sum, in0=births, in1=deaths,
                                        op=ALU.add)
                h2 = spool.tile([128, P], cdt, tag="h2")
                nc.vector.tensor_tensor(out=h2[:, :], in0=deaths, in1=births,
                                        op=ALU.subtract)
                nc.vector.scalar_tensor_tensor(
                    out=hL0, in0=pdim, scalar=-2.0, in1=h2[:, :],
                    op0=ALU.mult, op1=ALU.add)
                nc.vector.scalar_tensor_tensor(
                    out=hL1, in0=pdim, scalar=2.0, in1=h2[:, :],
                    op0=ALU.mult, op1=ALU.add)
                return hp

            def stage_compute(blk, hp):
                """Triangle construction + top-k + output for one block."""
                r0 = blk * 128
                s_b = hp[:, 0, :].rearrange("p (t q) -> p t q", t=1) \
                                 .broadcast_to([128, STEPS, P])
                # both landscapes' per-pair constants, broadcast over t
                hc_b = hp[:, 1:3, :].rearrange("p d (t q) -> p d t q", t=1) \
                                    .broadcast_to([128, N_DIMS, STEPS, P])

                # w = 2t - s on the tensor engine (two accumulating
                # identity matmuls per 512-col PSUM bank), aw = |w| on the
                # scalar engine reading PSUM; tri_d = h_d - aw on vector.
                TSEG = 4  # t-steps per PSUM bank (4*128 = 512 fp32)
                aw = bpool.tile([128, STEPS, P], cdt, tag="aw")
                trib = bpool.tile([128, N_DIMS, STEPS, P], cdt, tag="trib")
                tri0 = trib[:, 0, :, :]
                tri1 = trib[:, 1, :, :]
                if "construct" in ablate:
                    nc.vector.memset(trib[:, 0, 0, 0:8], 0.25)
                else:
                    for seg in range(STEPS // TSEG):
                        t0 = seg * TSEG
                        wp = ppool.tile([128, TSEG * P], F32)
                        wp3 = wp[:, :].rearrange("p (t q) -> p t q",
                                                 t=TSEG)
                        nc.tensor.matmul(
                            wp3, ident[:, :],
                            t2_rep[:, t0:t0 + TSEG, :],
                            start=True, stop=False)
                        nc.tensor.matmul(
                            wp3, nident[:, :],
                            s_b[:, t0:t0 + TSEG, :],
                            start=False, stop=True)
                        nc.scalar.activation(
                            aw[:, t0:t0 + TSEG, :], wp3, AF.Abs)
                    aw_b = aw[:, :, :].rearrange("p t q -> p (t q)") \
                                      .rearrange("p (d x) -> p d x", d=1) \
                                      .broadcast_to([128, N_DIMS, STEPS * P]) \
                                      .rearrange("p d (t q) -> p d t q",
                                                 t=STEPS)
                    nc.vector.tensor_tensor(out=trib[:, :, :, :], in0=hc_b,
                                            in1=aw_b, op=ALU.subtract)

                # top-8 over pairs per (dim, t); keep first two later
                top0 = spool.tile([128, STEPS, 8], cdt, tag="top0")
                top1 = spool.tile([128, STEPS, 8], cdt, tag="top1")
                if "max" in ablate:
                    nc.vector.memset(top0[:, 0, :], 0.25)
                    nc.vector.memset(top1[:, 0, :], 0.25)
                else:
                    for t in range(STEPS):
                        nc.vector.max(out=top0[:, t, :], in_=tri0[:, t, :])
                        nc.vector.max(out=top1[:, t, :], in_=tri1[:, t, :])


                # out row layout: (d, k, t); relu + un-double (scale 0.5);
                # land1 additionally drops its uniform +2 via bias -1.
                ot = spool.tile([128, N_DIMS * K_MAX * STEPS], F32, tag="ot")
                for d, top, bias in ((0, top0, 0.0), (1, top1, neg1[:, :])):
                    s = d * K_MAX * STEPS
                    nc.scalar.activation(
                        ot[:, s:s + K_MAX * STEPS].rearrange(
                            "p (k t) -> p k t", k=K_MAX),
                        top[:, :, 0:K_MAX].rearrange("p t k -> p k t"),
                        AF.Relu, scale=0.5, bias=bias)
                # issue from the ACT queue (same engine as the relu
                # producer) so the SP queue stays pure input-prefetch
                nc.scalar.dma_start(out=out_t[r0:r0 + 128, :], in_=ot[:, :])

            import contextlib
            loop_cm = (tc.For_i(0, repeat) if repeat > 1
                       else contextlib.nullcontext())
            with loop_cm:
                slots = {}
                for i in range(nb + 2):
                    if 1 <= i <= nb:
                        G, aux, G3 = slots[i - 1]
                        slots[i - 1] = (G, stage_fix(G, aux, G3))
                    if i < nb:
                        slots[i] = stage_load(i)
                    if i >= 2:
                        G, hp = slots.pop(i - 2)
                        stage_compute(i - 2, hp)
                    if i < nb:
                        G, aux = slots[i]
                        slots[i] = (G, aux, stage_g3copy(G))

    nc.compile()
    return nc


_NC_CACHE: dict = {}


def _get_nc(bc: int) -> bass.Bass:
    if bc not in _NC_CACHE:
        _NC_CACHE[bc] = build_nc(bc)
    return _NC_CACHE[bc]


def _host_index_prep(birth_idx, death_idx):
    """Re-encode per-row gather indices as an inverse table + feed arrays.

    Returns
      T     [R, HW] int16: T[r, pos] = slot (0..P2-1) whose index is pos
            (the lowest such slot = class winner), -1 if pos unreferenced.
      feeds [R, N_RANKS * P2] int16: feeds[r, (k-1)*P2 + winner] = slot of
            the rank-k member of the winner's collision class, -1 elsewhere.
    """
    q = np.concatenate([
        np.asarray(birth_idx).reshape(-1, P),
        np.asarray(death_idx).reshape(-1, P),
    ], axis=1).astype(np.int64)  # [R, P2]
    R = q.shape[0]
    rows = np.repeat(np.arange(R, dtype=np.int64), P2)
    slots = np.tile(np.arange(P2, dtype=np.int64), R)
    qf = q.reshape(-1)
    key = ((rows * HW + qf) << 9) | slots
    order = np.argsort(key)
    srows, sslots = rows[order], slots[order]
    sq = qf[order]
    kpos = key[order] >> 9
    same = np.empty(len(order), bool)
    same[0] = False
    same[1:] = kpos[1:] == kpos[:-1]

    T = np.full((R, HW), -1, np.int16)
    rank0 = ~same
    T[srows[rank0], sq[rank0]] = sslots[rank0]

    idx = np.arange(len(order))
    start_of = np.maximum.accumulate(np.where(same, 0, idx))
    occ = idx - start_of
    winner = sslots[start_of]
    feeds = np.full((R, N_RANKS * P2), -1, np.int16)
    for k in range(1, N_RANKS + 1):
        sel = occ == k
        feeds[srows[sel], (k - 1) * P2 + winner[sel]] = sslots[sel]
    assert occ.max() <= N_RANKS, (
        f"position collision class of size {occ.max() + 1} exceeds "
        f"N_RANKS={N_RANKS} feed ranks")
    return T, feeds


def make_in_maps(x, birth_idx, death_idx, pair_dim):
    x16 = np.asarray(x, dtype=np.float16).reshape(B, C, HW)
    T, feeds = _host_index_prep(birth_idx, death_idx)
    T = T.reshape(B, C, HW)
    aux = np.concatenate([
        feeds.reshape(B, C, N_RANKS * P2),
        np.asarray(pair_dim).reshape(B, C, P).astype(np.int16),
    ], axis=2)  # [B, C, N_RANKS*P2 + P]
    in_maps = []
    for core in range(N_CORES):
        b0, b1 = core * B_LOC, (core + 1) * B_LOC
        in_maps.append({
            "x16": np.ascontiguousarray(x16[b0:b1].reshape(BC_FULL, HW)),
            "tinv": np.ascontiguousarray(T[b0:b1].reshape(BC_FULL, HW)),
            "aux": np.ascontiguousarray(
                aux[b0:b1].reshape(BC_FULL, N_RANKS * P2 + P)),
        })
    return in_maps


def kernel(x, birth_idx, death_idx, pair_dim):
    x = np.asarray(x, dtype=np.float32)
    assert x.shape == (B, C, H, W)
    nc = _get_nc(BC_FULL)
    in_maps = make_in_maps(x, birth_idx, death_idx, pair_dim)
    res = run_bass_kernel_spmd(nc, in_maps, core_ids=list(range(N_CORES)))
    outs = [
        res.results[c]["out"].reshape(B_LOC, C, N_DIMS, K_MAX, STEPS)
        for c in range(N_CORES)
    ]
    return np.concatenate(outs, axis=0).astype(np.float32)

